# revision 1
# baseline (speedup 1.0000x reference)
"""Row-pair Trainium2 kernel for nn_BaseRVBackbone — v3.

Per dilated conv layer, output rows are processed in pairs (r, r+d)
(blocks of 2d rows; pair i of block b = rows (2db+i, 2db+i+d)).  Per pair
per column span, 6 matmuls accumulate BOTH output rows at once:
  inner (rhs = A-pair rows (r, r+d), K=128, M=128 = (out r | out r+d)):
    one mm per dw tap, rhs window shifted by the tap offset (free).
    lhsT blocks: (r->r: w[1]), (r->r+d: w[0]), (r+d->r: w[2]), (r+d->r+d: w[1])
  outer (rhs = C-pair rows (r-d, r+2d), half-zero lhsT):
    (r-d -> out r: w[0]), (r+2d -> out r+d: w[2])
ps[0:64] = out r, ps[64:128] = out r+d — combines are single-PSUM-operand
copies (conv1/conv2) or adds folding the residual (conv3').  No shifted
PSUM adds, so the NCC one-PSUM-operand rule is satisfied for free, and
GPSIMD (no PSUM access allowed) only does SBUF C-pair copies.

conv4 is folded twice: w3' = w3 @ W4c (x3 never materialized), and the
W4b x2 1x1 term lands in w3's center tap (w3'[1,1] += W4b).  Only the
W4a x1 term needs matmuls: 2 full-PE [128,128] mms per pair with
single-block zero-padded lhsT variants (mixing tile_position sub-tile
configs inside one PSUM accumulation group faults real hardware).

A build-time checker simulates every ring slot and asserts each read sees
exactly the row it expects.
"""

import sys

sys.path.insert(0, "/opt/trn_rl_repo")

import numpy as np
import ml_dtypes

import concourse.bacc as bacc
import concourse.mybir as mybir
import concourse.tile as tile
from concourse.bass_utils import run_bass_kernel_spmd

F32 = mybir.dt.float32
BF16 = mybir.dt.bfloat16
BF = ml_dtypes.bfloat16

B = 8
H = 48
WFULL = 2048
WC = 1024
CROP0 = 512
C = 64
NPER = 102400
PI = 3.14159
FOV_UP = 3.0 * PI / 180.0
FOV_DOWN = 25.0 * PI / 180.0
NPIX = H * WC

GP = 8
PW = WC + 2 * GP          # 1040
SPANS = [(0, 512), (512, WC)]
DIL = [1, 2, 3]
NP2 = 24                  # pairs per layer
NPC = 25                  # fpc slots (extra slot 24: row 47 at p0:64)
SA1, SC1 = 10, 6          # x1 A/C ring depths (pair slots)
SA2, SC2 = 10, 6          # x2 A/C ring depths
K2, K3 = 3, 8             # conv2 / conv3 step skews
XR = 6                    # xr row ring
NPIXP = NPIX + WC         # fimg padded with one zero row (strided loads)

# pair sequences (pair-start rows, in processing order)
Q1 = [2 * t for t in range(NP2)]                       # d=1: (r, r+1)
Q2 = [4 * (i // 2) + i % 2 for i in range(NP2)]        # d=2: (r, r+2)
Q3 = [6 * (i // 3) + i % 3 for i in range(NP2)]        # d=3: (r, r+3)


def own2(g):
    """x1 row g -> (conv2 A-pair start, half).  Pair (r2, r2+2)."""
    return (g, 0) if g % 4 in (0, 1) else (g - 2, 1)


def own3(g):
    """x2 row g -> (conv3 A-pair start, half).  Pair (r3, r3+3)."""
    return (g, 0) if g % 6 in (0, 1, 2) else (g - 3, 1)


I2 = {r2: i for i, r2 in enumerate(Q2)}   # pair start -> sequence index
I3 = {r3: i for i, r3 in enumerate(Q3)}


def _project(colored_points):
    import jax
    import jax.numpy as jnp

    cpu = jax.devices("cpu")[0]
    with jax.default_device(cpu):
        cp = jnp.asarray(colored_points)
        bi = cp[:, 0].astype(jnp.int32)
        xs, ys, zs = cp[:, 1], cp[:, 2], cp[:, 3]
        rs = jnp.sqrt(xs * xs + ys * ys + zs * zs)
        us = 0.5 * (1.0 - jnp.arctan2(ys, xs) / PI) * WFULL
        vs = (1.0 - (jnp.arcsin(zs / rs) + FOV_DOWN) / (FOV_UP + FOV_DOWN)) * H
        us = jnp.clip(us, 0, WFULL - 1).astype(jnp.int32)
        vs = jnp.clip(vs, 0, H - 1).astype(jnp.int32)
        return np.asarray(bi), np.asarray(us), np.asarray(vs)


def _prep_frame(pf, us, vs):
    n = us.shape[0]
    ordinals = np.arange(n)
    crop = (us >= CROP0) & (us < CROP0 + WC)
    pix = vs * WC + (us - CROP0)
    winner = np.full(NPIX, -1, np.int64)
    winner[pix[crop]] = ordinals[crop]
    occ = winner >= 0
    fimg = np.zeros((C, NPIXP), np.float32)   # one zero pad row at the end
    fimg[:, :NPIX][:, occ] = pf[winner[occ]].T
    return fimg.astype(BF), crop, pix


def _prep_weights(w1, w2, w3, w4):
    """lhsT packs: lin/lout [128, 3 layers * 3 dw * 128], w4dup [128, 128]."""
    w4m = np.asarray(w4, np.float32)[0, 0]
    w4a, w4b, w4c = w4m[0:64], w4m[64:128], w4m[128:192]
    w3f = np.einsum("hwij,jk->hwik", np.asarray(w3, np.float32), w4c)
    # fold the W4b x2 1x1 term into conv3's center tap: it is exactly a
    # (kh=1, kw=1) contribution, so no separate matmuls are needed for it
    w3f = w3f.copy()
    w3f[1, 1] += w4b
    layers = [np.asarray(w1, np.float32), np.asarray(w2, np.float32), w3f]

    lin = np.zeros((128, 9 * 128), np.float32)
    lout = np.zeros((128, 9 * 128), np.float32)
    for li, w in enumerate(layers):
        for kw in range(3):
            c0 = (li * 3 + kw) * 128
            lin[0:64, c0:c0 + 64] = w[1, kw]        # row r   -> out r
            lin[0:64, c0 + 64:c0 + 128] = w[0, kw]  # row r   -> out r+d
            lin[64:128, c0:c0 + 64] = w[2, kw]      # row r+d -> out r
            lin[64:128, c0 + 64:c0 + 128] = w[1, kw]
            lout[0:64, c0:c0 + 64] = w[0, kw]       # row r-d -> out r
            lout[64:128, c0 + 64:c0 + 128] = w[2, kw]  # row r+2d -> out r+d
    # W4a (x1) 1x1 lhsT: block-diag(W4a, W4a) — one full-PE matmul per
    # span against an (x1[r3] | x1[r3+3]) pair tile handles both out rows.
    w4x = np.zeros((128, 128), np.float32)
    w4x[0:64, 0:64] = w4a
    w4x[64:128, 64:128] = w4a
    return lin.astype(BF), lout.astype(BF), w4x.astype(BF)


_CACHED = {}


def _build():
    if "nc" in _CACHED:
        return _CACHED["nc"]
    nc = bacc.Bacc("TRN2", target_bir_lowering=False, debug=False,
                   enable_asserts=True, num_devices=B)
    fimg = nc.dram_tensor("fimg", [C, NPIXP], BF16, kind="ExternalInput").ap()
    wlin = nc.dram_tensor("wlin", [128, 1152], BF16, kind="ExternalInput").ap()
    wlout = nc.dram_tensor("wlout", [128, 1152], BF16, kind="ExternalInput").ap()
    w4d = nc.dram_tensor("w4d", [128, 128], BF16, kind="ExternalInput").ap()
    ximg = nc.dram_tensor("ximg", [C, NPIX], BF16, kind="ExternalOutput").ap()

    # ---- build-time ring content checker ----
    contents = {}   # (tile_name, slot, half) -> row id (or 'Z' for zeros)

    def put(tile_name, slot, half, row):
        contents[(tile_name, slot, half)] = row

    def get(tile_name, slot, half, want):
        got = contents.get((tile_name, slot, half), "?")
        assert got == want, (tile_name, slot, half, "want", want, "got", got)

    with tile.TileContext(nc) as tc:
        with tc.tile_pool(name="const", bufs=1) as cp:
            lint = cp.tile([128, 1152], BF16)
            nc.sync.dma_start(out=lint[:], in_=wlin)
            loutt = cp.tile([128, 1152], BF16)
            nc.sync.dma_start(out=loutt[:], in_=wlout)
            w4t = cp.tile([128, 128], BF16)
            scratch = cp.tile([128, 512], BF16)

            with tc.tile_pool(name="img", bufs=1) as ip, \
                 tc.tile_pool(name="ps", bufs=8, space="PSUM") as psp:
                fpa = ip.tile([128, NP2 * PW], BF16)   # A-slot t: (2t, 2t+1)
                fpc = ip.tile([128, NPC * PW], BF16)   # C-slot t: (2t-1, 2t+2)
                x1a = ip.tile([128, SA1 * PW], BF16)
                x1c = ip.tile([128, SC1 * PW], BF16)
                x2a = ip.tile([128, SA2 * PW], BF16)
                x2c = ip.tile([128, SC2 * PW], BF16)
                x13 = ip.tile([128, 3 * PW], BF16)  # (x1[r3] | x1[r3+3])
                xr = ip.tile([64, XR * WC], BF16)

                for t, ns in ((fpa, NP2), (fpc, NPC), (x1a, SA1),
                              (x1c, SC1), (x2a, SA2), (x2c, SC2), (x13, 3)):
                    v = t[:].rearrange("p (s w) -> p s w", s=ns)
                    nc.gpsimd.memset(v[:, :, 0:GP], 0.0)
                    nc.gpsimd.memset(v[:, :, PW - GP:PW], 0.0)
                # F boundary halves: fpc slot 0 low = row -1, slot 23 high = row 48
                nc.gpsimd.memset(fpc[0:64, 0 * PW + GP:0 * PW + GP + WC], 0.0)
                nc.gpsimd.memset(fpc[64:128, 23 * PW + GP:23 * PW + GP + WC], 0.0)

                # F loads: strided DMAs (every-2nd-row), small first chunks.
                # fimg has one zero pad row so 2k-row windows may overrun.
                fpav = fpa[:].rearrange("p (s w) -> p s w", s=NP2)
                fpcv = fpc[:].rearrange("p (s w) -> p s w", s=NPC)

                def fload(name, tile_v, dst_half, s0, s1, r_of_s):
                    """rows r_of_s(s) = 2s+const for s in [s0, s1) -> half."""
                    k = s1 - s0
                    if k <= 0:
                        return
                    r0 = r_of_s(s0)
                    src = fimg[:, r0 * WC:(r0 + 2 * k) * WC].rearrange(
                        "c (s v w) -> c s (v w)", s=k, v=2)[:, :, 0:WC]
                    p0, p1 = (0, 64) if dst_half == 0 else (64, 128)
                    nc.sync.dma_start(
                        out=tile_v[p0:p1, s0:s1, GP:GP + WC], in_=src)
                    for s in range(s0, s1):
                        put(name, s, dst_half, r_of_s(s))

                for (s0, s1) in ((0, 2), (2, 6), (6, 14), (14, 25)):
                    a1 = min(s1, NP2)
                    fload("fpa", fpav, 0, s0, a1, lambda s: 2 * s)
                    fload("fpa", fpav, 1, s0, a1, lambda s: 2 * s + 1)
                    # C low: row 2s-1 (slot 0 memset; slot 24 = row 47)
                    fload("fpc", fpcv, 0, max(s0, 1), s1, lambda s: 2 * s - 1)
                    # C high: row 2s+2 (slot 23 memset; slot 24 unused)
                    fload("fpc", fpcv, 1, s0, min(s1, 23), lambda s: 2 * s + 2)
                    if s0 == 0:
                        # w4t is first needed at step K3; defer its load past
                        # the first F chunks so conv1 starts sooner
                        nc.sync.dma_start(out=w4t[:], in_=w4d)
                put("fpc", 0, 0, -1)
                put("fpc", 23, 1, 48)

                # PE p-state warmup: burn the slow-clock ramp on dummy
                # matmuls while the first DMAs land (PE would be idle).
                # Memset on DVE so the warmup starts immediately.
                nc.vector.memset(scratch[:], 0.0)
                for wi in range(46):
                    wps = psp.tile([128, 512], F32, tag="ps")
                    nc.tensor.matmul(out=wps[:, 0:128], lhsT=scratch[:, 0:128],
                                     rhs=scratch[:, 0:128],
                                     start=True, stop=True)

                cp_rr = [0]

                def ps_copy(dst, src):
                    e = cp_rr[0] % 2
                    cp_rr[0] += 1
                    if e == 0:
                        nc.scalar.copy(out=dst, in_=src)
                    else:
                        nc.vector.tensor_copy(out=dst, in_=src)

                TSTEPS = NP2 + K3
                for t in range(TSTEPS):
                    # ---------- conv1: pair (2t, 2t+1) ----------
                    if t < NP2:
                        r = Q1[t]
                        get("fpa", t, 0, r)
                        get("fpa", t, 1, r + 1)
                        get("fpc", t, 0, r - 1)
                        get("fpc", t, 1, r + 2)
                        tiles = []
                        for c0, c1 in SPANS:
                            w = c1 - c0
                            ps = psp.tile([128, 512], F32, tag="ps")
                            for kw in range(3):
                                o = (kw - 1) * 1
                                cw = (0 * 3 + kw) * 128
                                a0 = t * PW + GP + c0 + o
                                nc.tensor.matmul(
                                    out=ps[:, 0:w], lhsT=lint[:, cw:cw + 128],
                                    rhs=fpa[:, a0:a0 + w],
                                    start=(kw == 0), stop=False)
                                nc.tensor.matmul(
                                    out=ps[:, 0:w], lhsT=loutt[:, cw:cw + 128],
                                    rhs=fpc[:, a0:a0 + w],
                                    start=False, stop=(kw == 2))
                            tiles.append(ps)
                        # combine copies -> x1a halves of owning pairs
                        for half, g in ((0, r), (1, r + 1)):
                            pr2, ph = own2(g)
                            sl = I2[pr2] % SA1
                            p0, p1 = (0, 64) if ph == 0 else (64, 128)
                            for (c0, c1), ps in zip(SPANS, tiles):
                                w = c1 - c0
                                dst = x1a[p0:p1,
                                          sl * PW + GP + c0:sl * PW + GP + c1]
                                ps_copy(dst, ps[64 * half:64 * half + 64, 0:w])
                            put("x1a", sl, ph, g)

                    # ---------- x1 C-copies for conv2 pair at t+1 ----------
                    i2n = t + 1 - K2
                    if 0 <= i2n < NP2:
                        r2 = Q2[i2n]
                        scl = i2n % SC1
                        for (half, g) in ((0, r2 - 2), (1, r2 + 4)):
                            p0, p1 = (0, 64) if half == 0 else (64, 128)
                            dst = x1c[p0:p1, scl * PW:(scl + 1) * PW]
                            if 0 <= g < H:
                                pr2, ph = own2(g)
                                sl = I2[pr2] % SA1
                                get("x1a", sl, ph, g)
                                q0, q1 = (0, 64) if ph == 0 else (64, 128)
                                nc.gpsimd.tensor_copy(
                                    out=dst,
                                    in_=x1a[q0:q1, sl * PW:(sl + 1) * PW])
                            else:
                                nc.gpsimd.memset(dst, 0.0)
                            put("x1c", scl, half, g if 0 <= g < H else "Z")

                    # ---------- conv2: pair (r2, r2+2) ----------
                    i2 = t - K2
                    if 0 <= i2 < NP2:
                        r2 = Q2[i2]
                        sA = I2[r2] % SA1
                        sC = i2 % SC1
                        get("x1a", sA, 0, r2)
                        get("x1a", sA, 1, r2 + 2)
                        get("x1c", sC, 0, r2 - 2 if r2 >= 2 else "Z")
                        get("x1c", sC, 1, r2 + 4 if r2 + 4 < H else "Z")
                        tiles = []
                        for c0, c1 in SPANS:
                            w = c1 - c0
                            ps = psp.tile([128, 512], F32, tag="ps")
                            for kw in range(3):
                                o = (kw - 1) * 2
                                cw = (1 * 3 + kw) * 128
                                a0 = sA * PW + GP + c0 + o
                                b0 = sC * PW + GP + c0 + o
                                nc.tensor.matmul(
                                    out=ps[:, 0:w], lhsT=lint[:, cw:cw + 128],
                                    rhs=x1a[:, a0:a0 + w],
                                    start=(kw == 0), stop=False)
                                nc.tensor.matmul(
                                    out=ps[:, 0:w], lhsT=loutt[:, cw:cw + 128],
                                    rhs=x1c[:, b0:b0 + w],
                                    start=False, stop=(kw == 2))
                            tiles.append(ps)
                        for half, g in ((0, r2), (1, r2 + 2)):
                            pr3, ph = own3(g)
                            sl = I3[pr3] % SA2
                            p0, p1 = (0, 64) if ph == 0 else (64, 128)
                            for (c0, c1), ps in zip(SPANS, tiles):
                                w = c1 - c0
                                dst = x2a[p0:p1,
                                          sl * PW + GP + c0:sl * PW + GP + c1]
                                ps_copy(dst, ps[64 * half:64 * half + 64, 0:w])
                            put("x2a", sl, ph, g)

                    # ---------- x2 C-copies for conv3 pair at t+1 ----------
                    i3n = t + 1 - K3
                    if 0 <= i3n < NP2:
                        r3 = Q3[i3n]
                        scl = i3n % SC2
                        for (half, g) in ((0, r3 - 3), (1, r3 + 6)):
                            p0, p1 = (0, 64) if half == 0 else (64, 128)
                            dst = x2c[p0:p1, scl * PW:(scl + 1) * PW]
                            if 0 <= g < H:
                                pr3, ph = own3(g)
                                sl = I3[pr3] % SA2
                                get("x2a", sl, ph, g)
                                q0, q1 = (0, 64) if ph == 0 else (64, 128)
                                nc.gpsimd.tensor_copy(
                                    out=dst,
                                    in_=x2a[q0:q1, sl * PW:(sl + 1) * PW])
                            else:
                                nc.gpsimd.memset(dst, 0.0)
                            put("x2c", scl, half, g if 0 <= g < H else "Z")

                    # ---------- x13 fill for conv3 pair at t+1 ----------
                    if 0 <= i3n < NP2:
                        r3n = Q3[i3n]
                        s13 = (i3n % 3) * PW
                        for (half, g) in ((0, r3n), (1, r3n + 3)):
                            pr2, ph = own2(g)
                            sl = I2[pr2] % SA1
                            get("x1a", sl, ph, g)
                            q0, q1 = (0, 64) if ph == 0 else (64, 128)
                            p0, p1 = (0, 64) if half == 0 else (64, 128)
                            eng = nc.scalar if half == 0 else nc.vector
                            (eng.copy if half == 0 else eng.tensor_copy)(
                                out=x13[p0:p1, s13:s13 + PW],
                                in_=x1a[q0:q1, sl * PW:(sl + 1) * PW])
                            put("x13", i3n % 3, half, g)

                    # ---------- conv3' + 1x1 + residual: pair (r3, r3+3) ----
                    i3 = t - K3
                    if 0 <= i3 < NP2:
                        r3 = Q3[i3]
                        sA = I3[r3] % SA2
                        sC = i3 % SC2
                        get("x2a", sA, 0, r3)
                        get("x2a", sA, 1, r3 + 3)
                        get("x2c", sC, 0, r3 - 3 if r3 >= 3 else "Z")
                        get("x2c", sC, 1, r3 + 6 if r3 + 6 < H else "Z")
                        get("x13", i3 % 3, 0, r3)
                        get("x13", i3 % 3, 1, r3 + 3)
                        s13 = (i3 % 3) * PW
                        tiles = []
                        for c0, c1 in SPANS:
                            w = c1 - c0
                            ps = psp.tile([128, 512], F32, tag="ps")
                            for kw in range(3):
                                o = (kw - 1) * 3
                                cw = (2 * 3 + kw) * 128
                                a0 = sA * PW + GP + c0 + o
                                b0 = sC * PW + GP + c0 + o
                                nc.tensor.matmul(
                                    out=ps[:, 0:w], lhsT=lint[:, cw:cw + 128],
                                    rhs=x2a[:, a0:a0 + w],
                                    start=(kw == 0), stop=False)
                                nc.tensor.matmul(
                                    out=ps[:, 0:w], lhsT=loutt[:, cw:cw + 128],
                                    rhs=x2c[:, b0:b0 + w],
                                    start=False, stop=False)
                            # x1 1x1 term via block-diag(W4a, W4a)
                            # against the (x1[r3] | x1[r3+3]) pair tile —
                            # one full-PE matmul covers both out rows.
                            # (W4b x2 is folded into conv3's center tap.)
                            ax = s13 + GP + c0
                            nc.tensor.matmul(
                                out=ps[:, 0:w], lhsT=w4t[:],
                                rhs=x13[:, ax:ax + w],
                                start=False, stop=True)
                            tiles.append(ps)
                        # combine adds (fold residual): xr row = ps + F row.
                        # F row g at partitions 0:64: even -> fpa low slot
                        # g/2, odd -> fpc low slot (g+1)/2 (incl. row 47 at
                        # fpc slot 24).  Keeps both SBUF operands at base
                        # partition 0 (NCC_IBIR297).
                        for oi, g in ((0, r3), (1, r3 + 3)):
                            o0 = 64 * oi
                            xsl = g % XR
                            if g % 2 == 0:
                                fb = (g // 2) * PW + GP
                                fsrc = fpa
                            else:
                                fb = ((g + 1) // 2) * PW + GP
                                fsrc = fpc
                            for (c0, c1), ps in zip(SPANS, tiles):
                                w = c1 - c0
                                nc.vector.tensor_add(
                                    out=xr[:, xsl * WC + c0:xsl * WC + c1],
                                    in0=ps[o0:o0 + 64, 0:w],
                                    in1=fsrc[0:64, fb + c0:fb + c1])
                            nc.sync.dma_start(
                                out=ximg[:, g * WC:(g + 1) * WC],
                                in_=xr[:, xsl * WC:(xsl + 1) * WC])
    nc.compile()
    _CACHED["nc"] = nc
    return nc


def _prepare_inmaps(colored_points, point_features, w1, w2, w3, w4):
    colored_points = np.ascontiguousarray(colored_points, np.float32)
    point_features = np.ascontiguousarray(point_features, np.float32)
    bi, us, vs = _project(colored_points)
    lin, lout, w4dup = _prep_weights(w1, w2, w3, w4)

    in_maps, crops, pixes = [], [], []
    for b in range(B):
        sl = slice(b * NPER, (b + 1) * NPER)
        fimg, crop, pix = _prep_frame(point_features[sl], us[sl], vs[sl])
        in_maps.append({"fimg": fimg, "wlin": lin, "wlout": lout,
                        "w4d": w4dup})
        crops.append(crop)
        pixes.append(pix)
    return in_maps, crops, pixes


def _expand(res, crops, pixes):
    outs = []
    for b in range(B):
        ximg = np.asarray(res.results[b]["ximg"]).astype(np.float32)
        ximg = ximg.reshape(C, NPIX)
        ob = np.zeros((NPER, C), np.float32)
        crop, pix = crops[b], pixes[b]
        ob[crop] = ximg[:, pix[crop]].T
        outs.append(ob)
    return np.concatenate(outs, axis=0)


def kernel(colored_points, point_features, w1, w2, w3, w4):
    in_maps, crops, pixes = _prepare_inmaps(
        colored_points, point_features, w1, w2, w3, w4)
    nc = _build()
    res = run_bass_kernel_spmd(nc, in_maps, core_ids=list(range(B)))
    return _expand(res, crops, pixes)


def run_traced(inputs):
    in_maps, _, _ = _prepare_inmaps(
        inputs["colored_points"], inputs["point_features"],
        inputs["w1"], inputs["w2"], inputs["w3"], inputs["w4"])
    nc = _build()
    return run_bass_kernel_spmd(nc, in_maps, core_ids=list(range(B)), trace=True)



# revision 21
# speedup vs baseline: 1.0866x; 1.0866x over previous
"""Row-pair Trainium2 kernel for nn_BaseRVBackbone — v4 (fp8 DoubleRow).

Structure is v3's row-pair scheme (see kernel_v1_backup.py), with every
matmul converted to fp8 DoubleRow instructions (2 independent K=128
groups per instruction at 0.5 cycles/row — 4x bf16 per logical matmul).

Precision plan (measured 1.48e-2 end-to-end in precsim.py --v2, vs the
2e-2 gate):
  - weights prescaled x8 and split: hi = e4m3(8W); lo = e5m2 pack of
    (8W - hi)*16 stored /16 (exponent shift exact in e5m2, no
    subnormals).  Effective weights ~bf16 quality.
  - fimg split on host: h = e4m3(f), l = e4m3((f-h)*16); conv1 runs
    3 products per logical matmul: (Wh,h), (Wlo,h), (Wh/16 as e5m2, l).
  - x1/x2 activations stored single e4m3 via the combine's x1/8 scaled
    copy (scalar.mul / vector.tensor_scalar_mul) -- psums carry 8x
    values from the weight prescale.
  - w4a 1x1: hi+lo both e4m3 (pair in ONE DoubleRow; lo suffers some
    subnormal loss, second-order).
  - residual: resid8 = bf16(8*front) DMA'd per-row into a small ring;
    final combine tensor_add(ps, resid8) -> xr bf16; ximg holds 8x
    values; host _expand divides by 8.

Group pairing into DoubleRow instructions (per pair-span: 9 conv1 +
6 conv2 + 7 conv3/w4a = 22 DR):
  conv1 hi : (ink0,ink1) (ink2,outk0) (outk1,outk2)      [e4m3 lhsT]
  conv1 lo : (loin_k, f16in_k) k=0..2 ; (loout_k, f16out_k) [e5m2]
  conv2/3 hi: (ink0,ink1) (ink2,outk0) (outk1,outk2)
  conv2/3 lo: (loin0,loin1) (loin2,loout0) (loout1,loout2)
  w4a      : (w4hi, w4lo) rhs stride 0                     [e4m3]
All groups address one F (fimg) or X (x1/x2) mega-tile, so arbitrary
constant strides between the two group windows are expressible as raw
APs.  A build-time checker still simulates every ring slot.
"""

import os as _os
import sys

sys.path.insert(0, "/opt/trn_rl_repo")

import numpy as np
import ml_dtypes

import concourse.bacc as bacc
import concourse.mybir as mybir
import concourse.tile as tile
from concourse.ap import AP
from concourse.bass_utils import run_bass_kernel_spmd

F32 = mybir.dt.float32
BF16 = mybir.dt.bfloat16
E4 = mybir.dt.float8e4
E5 = mybir.dt.float8e5
NE4 = ml_dtypes.float8_e4m3
NE5 = ml_dtypes.float8_e5m2
BF = ml_dtypes.bfloat16
DR = mybir.MatmulPerfMode.DoubleRow

B = 8
H = 48
WFULL = 2048
WC = 1024
CROP0 = 512
C = 64
NPER = 102400
PI = 3.14159
FOV_UP = 3.0 * PI / 180.0
FOV_DOWN = 25.0 * PI / 180.0
NPIX = H * WC

GP = 8
PW = WC + 2 * GP          # 1040
SPANS = [(0, 512), (512, WC)]
NP2 = 24                  # pairs per layer
SA1, SC1 = 10, 6          # x1 A/C ring depths (pair slots)
SA2, SC2 = 10, 6          # x2 A/C ring depths
K2, K3 = 3, 8             # conv2 / conv3 step skews
XR = 6                    # xr row ring
RR = 8                    # resid row ring
NPIXP = NPIX + WC         # fimg padded with one zero row (strided loads)

# F mega-tile: pair t occupies 4 consecutive slots [A_h, A_l, C_h, C_l]
# at 4t..4t+3 (keeps DoubleRow group strides within the 16-bit ISA
# step_elem field; h->l stride = 1 slot, A->C stride = 2 slots).
FQA_H, FQA_L, FQC_H, FQC_L = 0, 1, 2, 3    # q-index within a pair block
NFS = 96
# X tiles: x1 (a ring 10 + c ring 6), x2 (same), x13 (3 slots).
# Separate tiles so conv2/conv3 DoubleRow read-intervals cannot alias
# unrelated writes (false deps stall the PE otherwise).
X1A, X1C = 0, 10
X2A, X2C = 0, 10
X13 = 0
NX1S, NX2S, NX13S = 16, 16, 3

# Single all-e4m3 lhsT pack (mixing e4m3/e5m2 lhsT DoubleRow instructions
# inside one PSUM accumulation group faults the PE with real weight
# content — NRT_EXEC_UNIT_UNRECOVERABLE).  The global x64 weight prescale
# keeps the lo packs out of e4m3's subnormal zone.
GSC = 64.0                 # global weight prescale
LIN = 0                    # 9 blocks: (li*3+kw)*128
LOUT = 1152                # 9 blocks
W4H = 2304
W4LO = 2432
LOIN = 2560                # lo (residual) packs
LOOUT = 3712
F16IN = 4864               # 3 blocks (conv1 only): Wh/16
F16OUT = 5248
WE4W = 5632

# pair sequences (pair-start rows, in processing order)
Q1 = [2 * t for t in range(NP2)]                       # d=1: (r, r+1)
Q2 = [4 * (i // 2) + i % 2 for i in range(NP2)]        # d=2: (r, r+2)
Q3 = [6 * (i // 3) + i % 3 for i in range(NP2)]        # d=3: (r, r+3)


def own2(g):
    """x1 row g -> (conv2 A-pair start, half).  Pair (r2, r2+2)."""
    return (g, 0) if g % 4 in (0, 1) else (g - 2, 1)


def own3(g):
    """x2 row g -> (conv3 A-pair start, half).  Pair (r3, r3+3)."""
    return (g, 0) if g % 6 in (0, 1, 2) else (g - 3, 1)


I2 = {r2: i for i, r2 in enumerate(Q2)}   # pair start -> sequence index
I3 = {r3: i for i, r3 in enumerate(Q3)}


def _project(colored_points):
    import jax
    import jax.numpy as jnp

    cpu = jax.devices("cpu")[0]
    with jax.default_device(cpu):
        cp = jnp.asarray(colored_points)
        bi = cp[:, 0].astype(jnp.int32)
        xs, ys, zs = cp[:, 1], cp[:, 2], cp[:, 3]
        rs = jnp.sqrt(xs * xs + ys * ys + zs * zs)
        us = 0.5 * (1.0 - jnp.arctan2(ys, xs) / PI) * WFULL
        vs = (1.0 - (jnp.arcsin(zs / rs) + FOV_DOWN) / (FOV_UP + FOV_DOWN)) * H
        us = jnp.clip(us, 0, WFULL - 1).astype(jnp.int32)
        vs = jnp.clip(vs, 0, H - 1).astype(jnp.int32)
        return np.asarray(bi), np.asarray(us), np.asarray(vs)


def _prep_frame(pf, us, vs):
    n = us.shape[0]
    ordinals = np.arange(n)
    crop = (us >= CROP0) & (us < CROP0 + WC)
    pix = vs * WC + (us - CROP0)
    winner = np.full(NPIX, -1, np.int64)
    winner[pix[crop]] = ordinals[crop]
    occ = winner >= 0
    f = np.zeros((C, NPIXP), np.float32)   # one zero pad row at the end
    f[:, :NPIX][:, occ] = pf[winner[occ]].T
    fh = f.astype(NE4)
    fl = ((f - fh.astype(np.float32)) * 16.0).astype(NE4)
    residg = (f[:, :NPIX] * GSC).astype(BF)
    return fh, fl, residg, crop, pix


def _prep_weights(w1, w2, w3, w4):
    """Build the single WE4 [128, WE4W] e4m3 lhsT pack (x64 prescale)."""
    w4m = np.asarray(w4, np.float32)[0, 0]
    w4a, w4b, w4c = w4m[0:64], w4m[64:128], w4m[128:192]
    w3f = np.einsum("hwij,jk->hwik", np.asarray(w3, np.float32), w4c)
    w3f = w3f.copy()
    w3f[1, 1] += w4b          # W4b x2 1x1 folded into conv3 center tap
    layers = [np.asarray(w1, np.float32), np.asarray(w2, np.float32), w3f]

    ling = np.zeros((128, 1152), np.float32)
    loutg = np.zeros((128, 1152), np.float32)
    for li, w in enumerate(layers):
        w = w * GSC
        for kw in range(3):
            c0 = (li * 3 + kw) * 128
            ling[0:64, c0:c0 + 64] = w[1, kw]        # row r   -> out r
            ling[0:64, c0 + 64:c0 + 128] = w[0, kw]  # row r   -> out r+d
            ling[64:128, c0:c0 + 64] = w[2, kw]      # row r+d -> out r
            ling[64:128, c0 + 64:c0 + 128] = w[1, kw]
            loutg[0:64, c0:c0 + 64] = w[0, kw]       # row r-d -> out r
            loutg[64:128, c0 + 64:c0 + 128] = w[2, kw]  # r+2d -> out r+d

    lin_h = ling.astype(NE4)
    lout_h = loutg.astype(NE4)

    def lo_pack(wg, wh):
        lo16 = ((wg - wh.astype(np.float32)) * 16.0).astype(NE4)
        return (lo16.astype(np.float32) / 16.0).astype(NE4)

    lin_lo = lo_pack(ling, lin_h)
    lout_lo = lo_pack(loutg, lout_h)
    f16in = (lin_h[:, 0:384].astype(np.float32) / 16.0).astype(NE4)
    f16out = (lout_h[:, 0:384].astype(np.float32) / 16.0).astype(NE4)

    w4xg = np.zeros((128, 128), np.float32)
    w4xg[0:64, 0:64] = w4a * GSC
    w4xg[64:128, 64:128] = w4a * GSC
    w4h = w4xg.astype(NE4)
    w4lo = lo_pack(w4xg, w4h)

    we4 = np.zeros((128, WE4W), NE4)
    we4[:, LIN:LIN + 1152] = lin_h
    we4[:, LOUT:LOUT + 1152] = lout_h
    we4[:, W4H:W4H + 128] = w4h
    we4[:, W4LO:W4LO + 128] = w4lo
    we4[:, LOIN:LOIN + 1152] = lin_lo
    we4[:, LOOUT:LOOUT + 1152] = lout_lo
    we4[:, F16IN:F16IN + 384] = f16in
    we4[:, F16OUT:F16OUT + 384] = f16out
    return we4


def _mkap(t, off, dims):
    """Raw AP on tile t at element offset off with extra [stride, size]
    dims after the partition dim."""
    a = t[:]
    return AP(a.tensor, a.offset + off,
              [list(a.ap[0])] + [[s, n] for (s, n) in dims])


_CACHED = {}


def _build():
    if "nc" in _CACHED:
        return _CACHED["nc"]
    nc = bacc.Bacc("TRN2", target_bir_lowering=False, debug=False,
                   enable_asserts=True, num_devices=B)
    fimg_h = nc.dram_tensor("fimg_h", [C, NPIXP], E4, kind="ExternalInput").ap()
    fimg_l = nc.dram_tensor("fimg_l", [C, NPIXP], E4, kind="ExternalInput").ap()
    residg = nc.dram_tensor("residg", [C, NPIX], BF16, kind="ExternalInput").ap()
    we4d = nc.dram_tensor("we4", [128, WE4W], E4, kind="ExternalInput").ap()
    ximg = nc.dram_tensor("ximg", [C, NPIX], BF16, kind="ExternalOutput").ap()

    # ---- build-time ring content checker ----
    contents = {}   # (tile_name, slot, half) -> row id (or 'Z' for zeros)

    def put(tile_name, slot, half, row):
        contents[(tile_name, slot, half)] = row

    def get(tile_name, slot, half, want):
        got = contents.get((tile_name, slot, half), "?")
        assert got == want, (tile_name, slot, half, "want", want, "got", got)

    with tile.TileContext(nc) as tc:
        with tc.tile_pool(name="const", bufs=1) as cp:
            we4t = cp.tile([128, WE4W], E4)
            nc.sync.dma_start(out=we4t[:], in_=we4d)
            scratch = cp.tile([128, 512], BF16)

            with tc.tile_pool(name="img", bufs=1) as ip, \
                 tc.tile_pool(name="ps", bufs=8, space="PSUM") as psp:
                F = ip.tile([128, NFS * PW], E4)
                X1 = ip.tile([128, NX1S * PW], E4)
                X2 = ip.tile([128, NX2S * PW], E4)
                X3 = ip.tile([128, NX13S * PW], E4)
                xr = ip.tile([64, XR * WC], BF16)
                rring = ip.tile([64, RR * WC], BF16)

                # [p, q(variant), s(pair), w] view of F
                Fq = F[:].rearrange("p (s q w) -> p q s w", s=NP2, q=4)
                Fv = F[:].rearrange("p (s w) -> p s w", s=NFS)
                nc.gpsimd.memset(Fv[:, :, 0:GP], 0.0)
                nc.gpsimd.memset(Fv[:, :, PW - GP:PW], 0.0)
                for _xt, _ns in ((X1, NX1S), (X2, NX2S), (X3, NX13S)):
                    _xv = _xt[:].rearrange("p (s w) -> p s w", s=_ns)
                    nc.gpsimd.memset(_xv[:, :, 0:GP], 0.0)
                    nc.gpsimd.memset(_xv[:, :, PW - GP:PW], 0.0)
                # F boundary halves: C pair 0 low = row -1, pair 23 high =
                # row 48 (both h and l variants)
                for q in (FQC_H, FQC_L):
                    s0 = (0 * 4 + q) * PW
                    nc.gpsimd.memset(F[0:64, s0 + GP:s0 + GP + WC], 0.0)
                    s23 = (23 * 4 + q) * PW
                    nc.gpsimd.memset(F[64:128, s23 + GP:s23 + GP + WC], 0.0)

                def fload(name, src, q, dst_half, s0, s1, r_of_s):
                    """rows r_of_s(s) = 2s+const for s in [s0, s1) -> half."""
                    k = s1 - s0
                    if k <= 0:
                        return
                    r0 = r_of_s(s0)
                    sv = src[:, r0 * WC:(r0 + 2 * k) * WC].rearrange(
                        "c (s v w) -> c s (v w)", s=k, v=2)[:, :, 0:WC]
                    p0, p1 = (0, 64) if dst_half == 0 else (64, 128)
                    nc.sync.dma_start(
                        out=Fq[p0:p1, q, s0:s1, GP:GP + WC], in_=sv)
                    for s in range(s0, s1):
                        put(name, s, dst_half, r_of_s(s))

                for (s0, s1) in ((0, 2), (2, 6), (6, 14), (14, 24)):
                    for (nm, src, aq, cq) in (
                            ("fa_h", fimg_h, FQA_H, FQC_H),
                            ("fa_l", fimg_l, FQA_L, FQC_L)):
                        cnm = "fc" + nm[2:]
                        fload(nm, src, aq, 0, s0, s1, lambda s: 2 * s)
                        fload(nm, src, aq, 1, s0, s1, lambda s: 2 * s + 1)
                        # C low: row 2s-1 (pair-0 low is the memset row -1)
                        fload(cnm, src, cq, 0, max(s0, 1), s1,
                              lambda s: 2 * s - 1)
                        # C high: row 2s+2 (pair-23 high is the memset row 48)
                        fload(cnm, src, cq, 1, s0, min(s1, 23),
                              lambda s: 2 * s + 2)
                for nm in ("fc_h", "fc_l"):
                    put(nm, 0, 0, -1)
                    put(nm, 23, 1, 48)

                # PE p-state warmup: burn the slow-clock ramp on dummy
                # matmuls while the first DMAs land.
                nc.vector.memset(scratch[:], 0.0)
                for wi in range(46):
                    wps = psp.tile([128, 512], F32, tag="ps")
                    nc.tensor.matmul(out=wps[:, 0:128], lhsT=scratch[:, 0:128],
                                     rhs=scratch[:, 0:128],
                                     start=True, stop=True)

                cp_rr = [0]

                def ps_scaled_copy(dst, src):
                    """dst = src / GSC (undo the weight prescale)."""
                    e = cp_rr[0] % 2
                    cp_rr[0] += 1
                    if e == 0:
                        nc.scalar.mul(dst, src, 1.0 / GSC)
                    else:
                        nc.vector.tensor_scalar_mul(dst, src, 1.0 / GSC)

                DBG_SKIP = set(_os.environ.get("KDBG_SKIP", "").split(","))

                def conv_span(ps, rt, li, d, aslot, cslot, c0, w,
                              extra_w4=None):
                    """Emit the DR group for one conv layer on one span.
                    aslot/cslot are slot indices in rt's mega-tile; li is
                    the layer (lhsT block row); d the dilation.
                    extra_w4 = x13 slot for the conv3 w4a product."""
                    A = aslot * PW + GP + c0
                    Cc = cslot * PW + GP + c0
                    L = LIN + li * 384
                    Lo = LOUT + li * 384
                    g = []
                    # hi (e4m3)
                    if f"hi{li}" not in DBG_SKIP:
                        g.append((we4t, L, 128, A - d, d))
                        g.append((we4t, L + 256, Lo - (L + 256), A + d,
                                  (Cc - d) - (A + d)))
                        g.append((we4t, Lo + 128, 128, Cc, d))
                    # lo (weight-residual packs, e4m3)
                    if f"lo{li}" not in DBG_SKIP:
                        if li == 0:
                            # conv1: pair (loin_k on h) with (f16in_k on l);
                            # h->l is the adjacent slot (stride PW)
                            for kw in range(3):
                                o = (kw - 1) * d
                                g.append((we4t, LOIN + kw * 128,
                                          F16IN - LOIN, A + o, PW))
                            for kw in range(3):
                                o = (kw - 1) * d
                                g.append((we4t, LOOUT + kw * 128,
                                          F16OUT - LOOUT, Cc + o, PW))
                        else:
                            Lil = LOIN + li * 384
                            Lol = LOOUT + li * 384
                            g.append((we4t, Lil, 128, A - d, d))
                            g.append((we4t, Lil + 256, Lol - (Lil + 256),
                                      A + d, (Cc - d) - (A + d)))
                            g.append((we4t, Lol + 128, 128, Cc, d))
                    # w4a LAST: a stride-0 rhs DoubleRow faults the PE when
                    # another matmul follows it in the accumulation group;
                    # as the final instruction it executes correctly.
                    if extra_w4 is not None and "w4a" not in DBG_SKIP:
                        x13o = extra_w4 * PW + GP + c0
                        g.append((we4t, W4H, 128, x13o, 0, X3))
                    for i, ins in enumerate(g):
                        wt, l0, ls, r0, rs = ins[:5]
                        rtile = ins[5] if len(ins) > 5 else rt
                        lh = _mkap(wt, l0, [(ls, 2), (1, 128)])
                        rh = _mkap(rtile, r0, [(rs, 2), (1, w)])
                        nc.tensor.matmul(out=ps[:, 0:w], lhsT=lh, rhs=rh,
                                         perf_mode=DR, start=(i == 0),
                                         stop=(i == len(g) - 1))

                TSTEPS = min(NP2 + K3,
                             int(_os.environ.get("KDBG_TSTEPS", NP2 + K3)))
                for t in range(TSTEPS):
                    # ---------- conv1: pair (2t, 2t+1) ----------
                    if t < NP2:
                        r = Q1[t]
                        for nm in ("fa_h", "fa_l"):
                            get(nm, t, 0, r)
                            get(nm, t, 1, r + 1)
                        for nm in ("fc_h", "fc_l"):
                            get(nm, t, 0, r - 1)
                            get(nm, t, 1, r + 2)
                        tiles = []
                        for c0, c1 in SPANS:
                            w = c1 - c0
                            ps = psp.tile([128, 512], F32, tag="ps")
                            conv_span(ps, F, 0, 1, 4 * t + FQA_H,
                                      4 * t + FQC_H, c0, w)
                            tiles.append(ps)
                        # combine scaled copies -> x1a halves of owning pairs
                        for half, g in ((0, r), (1, r + 1)):
                            pr2, ph = own2(g)
                            sl = X1A + I2[pr2] % SA1
                            p0, p1 = (0, 64) if ph == 0 else (64, 128)
                            for (c0, c1), ps in zip(SPANS, tiles):
                                w = c1 - c0
                                dst = X1[p0:p1,
                                         sl * PW + GP + c0:sl * PW + GP + c1]
                                ps_scaled_copy(dst,
                                               ps[64 * half:64 * half + 64,
                                                  0:w])
                            put("x1a", sl, ph, g)

                    # ---------- x1 C-copies for conv2 pair at t+1 ----------
                    i2n = t + 1 - K2
                    if 0 <= i2n < NP2:
                        r2 = Q2[i2n]
                        scl = X1C + i2n % SC1
                        for (half, g) in ((0, r2 - 2), (1, r2 + 4)):
                            p0, p1 = (0, 64) if half == 0 else (64, 128)
                            dst = X1[p0:p1, scl * PW:(scl + 1) * PW]
                            if 0 <= g < H:
                                pr2, ph = own2(g)
                                sl = X1A + I2[pr2] % SA1
                                get("x1a", sl, ph, g)
                                q0, q1 = (0, 64) if ph == 0 else (64, 128)
                                nc.gpsimd.tensor_copy(
                                    out=dst,
                                    in_=X1[q0:q1, sl * PW:(sl + 1) * PW])
                            else:
                                nc.gpsimd.memset(dst, 0.0)
                            put("x1c", scl, half, g if 0 <= g < H else "Z")

                    # ---------- conv2: pair (r2, r2+2) ----------
                    i2 = t - K2
                    if 0 <= i2 < NP2:
                        r2 = Q2[i2]
                        sA = X1A + I2[r2] % SA1
                        sC = X1C + i2 % SC1
                        get("x1a", sA, 0, r2)
                        get("x1a", sA, 1, r2 + 2)
                        get("x1c", sC, 0, r2 - 2 if r2 >= 2 else "Z")
                        get("x1c", sC, 1, r2 + 4 if r2 + 4 < H else "Z")
                        tiles = []
                        for c0, c1 in SPANS:
                            w = c1 - c0
                            ps = psp.tile([128, 512], F32, tag="ps")
                            conv_span(ps, X1, 1, 2, sA, sC, c0, w)
                            tiles.append(ps)
                        for half, g in ((0, r2), (1, r2 + 2)):
                            pr3, ph = own3(g)
                            sl = X2A + I3[pr3] % SA2
                            p0, p1 = (0, 64) if ph == 0 else (64, 128)
                            for (c0, c1), ps in zip(SPANS, tiles):
                                w = c1 - c0
                                dst = X2[p0:p1,
                                         sl * PW + GP + c0:sl * PW + GP + c1]
                                ps_scaled_copy(dst,
                                               ps[64 * half:64 * half + 64,
                                                  0:w])
                            put("x2a", sl, ph, g)

                    # ---------- x2 C-copies for conv3 pair at t+1 ----------
                    i3n = t + 1 - K3
                    if 0 <= i3n < NP2:
                        r3 = Q3[i3n]
                        scl = X2C + i3n % SC2
                        for (half, g) in ((0, r3 - 3), (1, r3 + 6)):
                            p0, p1 = (0, 64) if half == 0 else (64, 128)
                            dst = X2[p0:p1, scl * PW:(scl + 1) * PW]
                            if 0 <= g < H:
                                pr3, ph = own3(g)
                                sl = X2A + I3[pr3] % SA2
                                get("x2a", sl, ph, g)
                                q0, q1 = (0, 64) if ph == 0 else (64, 128)
                                nc.gpsimd.tensor_copy(
                                    out=dst,
                                    in_=X2[q0:q1, sl * PW:(sl + 1) * PW])
                            else:
                                nc.gpsimd.memset(dst, 0.0)
                            put("x2c", scl, half, g if 0 <= g < H else "Z")

                    # ---------- x13 fill + resid loads for pair at t+1 -----
                    if 0 <= i3n < NP2:
                        r3n = Q3[i3n]
                        s13 = (X13 + i3n % 3) * PW
                        for (half, g) in ((0, r3n), (1, r3n + 3)):
                            pr2, ph = own2(g)
                            sl = X1A + I2[pr2] % SA1
                            get("x1a", sl, ph, g)
                            q0, q1 = (0, 64) if ph == 0 else (64, 128)
                            p0, p1 = (0, 64) if half == 0 else (64, 128)
                            eng = nc.scalar if half == 0 else nc.vector
                            (eng.copy if half == 0 else eng.tensor_copy)(
                                out=X3[p0:p1, s13:s13 + PW],
                                in_=X1[q0:q1, sl * PW:(sl + 1) * PW])
                            put("x13", X13 + i3n % 3, half, g)
                            rsl = g % RR
                            nc.sync.dma_start(
                                out=rring[:, rsl * WC:(rsl + 1) * WC],
                                in_=residg[:, g * WC:(g + 1) * WC])
                            put("rr", rsl, 0, g)

                    # ---------- conv3' + w4a + residual: pair (r3, r3+3) ---
                    i3 = t - K3
                    if 0 <= i3 < NP2:
                        r3 = Q3[i3]
                        sA = X2A + I3[r3] % SA2
                        sC = X2C + i3 % SC2
                        get("x2a", sA, 0, r3)
                        get("x2a", sA, 1, r3 + 3)
                        get("x2c", sC, 0, r3 - 3 if r3 >= 3 else "Z")
                        get("x2c", sC, 1, r3 + 6 if r3 + 6 < H else "Z")
                        get("x13", X13 + i3 % 3, 0, r3)
                        get("x13", X13 + i3 % 3, 1, r3 + 3)
                        tiles = []
                        for c0, c1 in SPANS:
                            w = c1 - c0
                            ps = psp.tile([128, 512], F32, tag="ps")
                            conv_span(ps, X2, 2, 3, sA, sC, c0, w,
                                      extra_w4=X13 + i3 % 3)
                            tiles.append(ps)
                        # combine adds: xr row = ps + resid8 row (both 8x).
                        for oi, g in ((0, r3), (1, r3 + 3)):
                            o0 = 64 * oi
                            xsl = g % XR
                            rsl = g % RR
                            get("rr", rsl, 0, g)
                            for (c0, c1), ps in zip(SPANS, tiles):
                                w = c1 - c0
                                nc.vector.tensor_add(
                                    out=xr[:, xsl * WC + c0:xsl * WC + c1],
                                    in0=ps[o0:o0 + 64, 0:w],
                                    in1=rring[:, rsl * WC + c0:rsl * WC + c1])
                            nc.sync.dma_start(
                                out=ximg[:, g * WC:(g + 1) * WC],
                                in_=xr[:, xsl * WC:(xsl + 1) * WC])
    nc.compile()
    _CACHED["nc"] = nc
    return nc


def _prepare_inmaps(colored_points, point_features, w1, w2, w3, w4):
    colored_points = np.ascontiguousarray(colored_points, np.float32)
    point_features = np.ascontiguousarray(point_features, np.float32)
    bi, us, vs = _project(colored_points)
    we4 = _prep_weights(w1, w2, w3, w4)

    in_maps, crops, pixes = [], [], []
    for b in range(B):
        sl = slice(b * NPER, (b + 1) * NPER)
        fh, fl, residg, crop, pix = _prep_frame(point_features[sl], us[sl],
                                                vs[sl])
        in_maps.append({"fimg_h": fh, "fimg_l": fl, "residg": residg,
                        "we4": we4})
        crops.append(crop)
        pixes.append(pix)
    return in_maps, crops, pixes


def _expand(res, crops, pixes):
    outs = []
    for b in range(B):
        ximg = np.asarray(res.results[b]["ximg"]).astype(np.float32)
        ximg = ximg.reshape(C, NPIX) * (1.0 / GSC)
        ob = np.zeros((NPER, C), np.float32)
        crop, pix = crops[b], pixes[b]
        ob[crop] = ximg[:, pix[crop]].T
        outs.append(ob)
    return np.concatenate(outs, axis=0)


def kernel(colored_points, point_features, w1, w2, w3, w4):
    in_maps, crops, pixes = _prepare_inmaps(
        colored_points, point_features, w1, w2, w3, w4)
    nc = _build()
    res = run_bass_kernel_spmd(nc, in_maps, core_ids=list(range(B)))
    return _expand(res, crops, pixes)


def run_traced(inputs):
    in_maps, _, _ = _prepare_inmaps(
        inputs["colored_points"], inputs["point_features"],
        inputs["w1"], inputs["w2"], inputs["w3"], inputs["w4"])
    nc = _build()
    return run_bass_kernel_spmd(nc, in_maps, core_ids=list(range(B)),
                                trace=True)
